# revision 1
# baseline (speedup 1.0000x reference)
# DeepESN Trainium2 kernel: 3-layer leaky-integrator ESN (leaky=1.0).
#   h_t = tanh(x_t @ Win + h_{t-1} @ Wrec + b), outputs concatenated over layers.
# Sharding: data-parallel over batch (16 seqs -> 2 per core on 8 cores).
# Per core: serial over layers; per layer: input projection runs one
# 128-step chunk ahead of the sequential scan; scan keeps state transposed
# (hT as 4x[128,BL] chunks) so each step is 16 Wrec-stationary matmuls
# + identity-matmul pre-injection + per-chunk tanh, with no transposes.

import os
import numpy as np

B, T, I, U, L = 16, 2048, 128, 512, 3
NCORES = 8
BL = B // NCORES          # 2 sequences per core
CH = 128                  # steps per chunk (ring size)
KC = U // 128             # 4 unit chunks
P = 128

_cache = {}


def _patch_ldwopt():
    import concourse.bass_utils as bu
    if getattr(bu, "_ldwopt_patched", False):
        return
    orig = bu.run_command

    def patched(argv, **kw):
        argv = ["--enable-ldw-opt=true" if a == "--enable-ldw-opt=false"
                else a for a in argv]
        return orig(argv, **kw)

    bu.run_command = patched
    bu._ldwopt_patched = True


def _build(T_, mm_fp16):
    if os.environ.get("DEEPESN_LDWOPT", "0") == "1":
        _patch_ldwopt()
    import concourse.bass as bass
    import concourse.tile as tile
    import concourse.mybir as mybir
    from concourse.vector_clock import ScopedClock

    fp32 = mybir.dt.float32
    mm_dt = mybir.dt.float16 if mm_fp16 else fp32
    AF = mybir.ActivationFunctionType
    PADT = T_ + CH
    NCHUNK = T_ // CH

    import bass_rust

    def split_excess_waits(nc):
        # This walrus build accepts at most ONE sync-wait per instruction;
        # Tile's scheduler can assign several. Move the excess onto NoOp
        # carriers inserted just before, on the same engine sequencer.
        n = 0
        for f in nc.m.functions:
            for bb in f.blocks:
                il = bb.instructions
                new_il = []
                for inst in il:
                    si = inst.sync_info
                    if si is not None and len(si.on_wait) > 1:
                        waits = list(si.on_wait)
                        si.on_wait.clear()
                        si.on_wait.append(waits[-1])
                        for w in waits[:-1]:
                            nop = mybir.InstNoOp(
                                name=f"wsp{n}", ins=[], outs=[])
                            n += 1
                            nop.engine = inst.engine
                            nop.sync_info = bass_rust.SyncInfo(
                                on_wait=[w], on_update=[])
                            new_il.append(nop)
                    new_il.append(inst)
                bb.instructions = new_il
        return n

    PatchedTC = tile.TileContext

    import concourse.bacc as bacc
    nc = bacc.Bacc()
    ds = bass.ds

    x_in = nc.declare_dram_parameter("x", [BL, PADT, I], fp32, isOutput=False)
    win_in = [
        nc.declare_dram_parameter(f"Win{l}", [I if l == 0 else U, U], fp32,
                                  isOutput=False)
        for l in range(L)
    ]
    wrec_in = [
        nc.declare_dram_parameter(f"Wrec{l}", [U, U], fp32, isOutput=False)
        for l in range(L)
    ]
    b_in = [
        nc.declare_dram_parameter(f"b{l}", [U], fp32, isOutput=False)
        for l in range(L)
    ]
    id_in = nc.declare_dram_parameter("ident", [P, P], fp32, isOutput=False)
    out = nc.declare_dram_parameter("out", [BL, T_, L * U], fp32, isOutput=True)

    with PatchedTC(nc) as tc, \
         tc.tile_pool(name="consts", bufs=1) as consts, \
         tc.tile_pool(name="state", bufs=1) as state, \
         tc.tile_pool(name="xr", bufs=2) as xr_pool, \
         tc.tile_pool(name="xt", bufs=2) as xt_pool, \
         tc.tile_pool(name="hrow", bufs=2) as hrow_pool, \
         tc.tile_pool(name="zps", bufs=1, space="PSUM") as zps_pool, \
         tc.tile_pool(name="pps", bufs=2, space="PSUM") as pps_pool, \
         tc.tile_pool(name="tps", bufs=2, space="PSUM") as tps_pool:

        ident = consts.tile([P, P], fp32, tag="ident", name="ident")
        nc.sync.dma_start(out=ident, in_=id_in[:, :])

        W_sb, Win_sb, bias_sb = [], [], []
        for l in range(L):
            w = consts.tile([P, KC, U], mm_dt, tag=f"wrec{l}", name=f"wrec{l}")
            nc.sync.dma_start(
                out=w, in_=wrec_in[l].rearrange("(kc p) u -> p kc u", p=P))
            W_sb.append(w)
            ikc = 1 if l == 0 else KC
            wi = consts.tile([P, ikc, U], fp32, tag=f"win{l}", name=f"win{l}")
            nc.sync.dma_start(
                out=wi, in_=win_in[l].rearrange("(kc p) u -> p kc u", p=P))
            Win_sb.append(wi)
            bb = consts.tile([P, KC], fp32, tag=f"b{l}", name=f"bsb{l}")
            nc.sync.dma_start(
                out=bb, in_=b_in[l].rearrange("(mc p) -> p mc", p=P))
            bias_sb.append(bb)

        # scan state ring: ring[p, slot, kc, b, t] = h[b, t0+t, kc*128+p]
        ring = state.tile([P, 2, KC, BL, CH], mm_dt, tag="ring", name="ring")
        # pre-activation ring, same slotting: preT[p, slot, mc, b, t]
        preT = state.tile([P, 2, KC, BL, CH], fp32, tag="preT", name="preT")
        # full-layer hT for next layer's projection (ping-pong by layer)
        hT_ab = [
            state.tile([P, KC, BL, PADT], mm_dt, tag=f"hT{i}", name=f"hT{i}") for i in range(2)
        ]
        for i in range(2):
            # the final in-loop projection reads one chunk past T (its
            # result is never used); keep that pad region initialized
            nc.vector.memset(hT_ab[i][:, :, :, T_:], 0.0)

        def project(l, t0n, sn):
            """Fill preT slot sn with pre[b, t0n:t0n+CH, :] for layer l."""
            if l == 0:
                xT_blk = xt_pool.tile([P, BL, CH], fp32, tag="xT", name="xT")
                for b in range(BL):
                    xr = xr_pool.tile([P, I], fp32, tag="xr", name="xr")
                    nc.sync.dma_start(out=xr, in_=x_in[b, ds(t0n, CH), :])
                    xt_ps = tps_pool.tile([P, P], fp32, tag="tps", name="xtps")
                    nc.tensor.transpose(xt_ps, xr, ident)
                    nc.vector.tensor_copy(xT_blk[:, b, :], xt_ps)
                for mc in range(KC):
                    pp = pps_pool.tile([P, BL, CH], fp32, tag="pp", name="pp")
                    nc.tensor.matmul(
                        pp, Win_sb[0][:, 0, mc * P:(mc + 1) * P],
                        xT_blk[:, :, :], start=True, stop=True)
                    nc.vector.tensor_scalar_add(
                        preT[:, sn, mc, :, :], pp, bias_sb[0][:, mc:mc + 1])
            else:
                hprev = hT_ab[(l + 1) % 2]
                for mc in range(KC):
                    pp = pps_pool.tile([P, BL, CH], fp32, tag="pp", name="pp")
                    for kc in range(KC):
                        nc.tensor.matmul(
                            pp, Win_sb[l][:, kc, mc * P:(mc + 1) * P],
                            hprev[:, kc, :, ds(t0n, CH)],
                            start=(kc == 0), stop=(kc == KC - 1))
                    nc.vector.tensor_scalar_add(
                        preT[:, sn, mc, :, :], pp, bias_sb[l][:, mc:mc + 1])

        nochain = os.environ.get("DEEPESN_NOCHAIN", "0") == "1"
        act1 = os.environ.get("DEEPESN_ACT1", "0") == "1"
        opta = os.environ.get("DEEPESN_OPTA", "0") == "1"
        dummy = state.tile([P, 2, KC, BL], mm_dt, tag="dummy", name="dummy")

        if opta:
            vps = [zps_pool.tile([BL, U], fp32, tag=f"vps{i}", name=f"vps{i}")
                   for i in range(2)]
            vsb = [state.tile([BL, U], fp32, tag=f"vsb{i}", name=f"vsb{i}")
                   for i in range(2)]

        def scan_chunk_opta(l, s, ps, zs):
            # h-stationary orientation: v = h_{t-1} @ Wrec as [BL, 512]
            # (2-column weight loads), then 4 tiny identity-matmuls fold
            # v back into the transposed ring orientation, accumulating
            # on top of the pre-injection.
            for u in range(CH):
                zp = zs[u % 2]
                vp = vps[u % 2]
                v_sb = vsb[u % 2]
                for kc in range(KC):
                    if u > 0:
                        hprev = ring[:, s, kc, :, u - 1]
                    else:
                        hprev = ring[:, ps, kc, :, CH - 1]
                    nc.tensor.matmul(
                        vp, hprev, W_sb[l][:, kc, :],
                        start=(kc == 0), stop=(kc == KC - 1))
                nc.vector.tensor_copy(v_sb, vp)
                nc.tensor.matmul(
                    zp[:, :, :], ident, preT[:, s, :, :, u],
                    start=True, stop=True)
                for mc in range(KC):
                    nc.tensor.matmul(
                        zp[:, mc, :], v_sb[:, mc * P:(mc + 1) * P],
                        ident[0:2, 0:2], start=False, stop=True,
                        skip_group_check=True)
                    nc.scalar.activation(
                        ring[:, s, mc, :, u], zp[:, mc, :], AF.Tanh)

        def scan_chunk(l, s, ps, zs):
            if opta:
                return scan_chunk_opta(l, s, ps, zs)
            for u in range(CH):
                zp = zs[u % 4]
                # stop=True closes the sim's psum group-tracking flag
                # immediately (stop is a no-op on hardware); the Wrec MMs
                # below accumulate via per-element has_written bits.
                nc.tensor.matmul(
                    zp[:, :, :], ident, preT[:, s, :, :, u],
                    start=True, stop=True)
                for mc in range(KC):
                    for kc in range(KC):
                        if u > 0:
                            rhs = ring[:, s, kc, :, u - 1]
                        else:
                            rhs = ring[:, ps, kc, :, CH - 1]
                        nc.tensor.matmul(
                            zp[:, mc, :], W_sb[l][:, kc, mc * P:(mc + 1) * P],
                            rhs, start=False, stop=(kc == KC - 1),
                            skip_group_check=True)
                    if act1:
                        continue
                    if nochain:
                        # timing experiment: break the ACT->MM dependency
                        nc.scalar.activation(
                            dummy[:, u % 2, mc, :], zp[:, mc, :], AF.Tanh)
                    else:
                        nc.scalar.activation(
                            ring[:, s, mc, :, u], zp[:, mc, :], AF.Tanh)
                if act1:
                    nc.scalar.activation(
                        ring[:, s, :, :, u], zp[:, :, :], AF.Tanh)

        def writeout(l, s, t0):
            for b in range(BL):
                h_rows = hrow_pool.tile([P, U], fp32, tag="hrow", name="hrow")
                for kc in range(KC):
                    hp = tps_pool.tile([P, P], mm_dt, tag="tps", name="htps")
                    nc.tensor.transpose(hp, ring[:, s, kc, b, :], ident)
                    nc.vector.tensor_copy(h_rows[:, kc * P:(kc + 1) * P], hp)
                nc.sync.dma_start(
                    out=out[b, ds(t0, CH), l * U:(l + 1) * U], in_=h_rows)
            if l < L - 1:
                nc.sync.dma_start(
                    out=hT_ab[l % 2][:, :, :, ds(t0, CH)],
                    in_=ring[:, s, :, :, :])

        def whole_kernel():
            for l in range(L):
                zs = [zps_pool.tile([P, KC, BL], fp32, tag=f"z{i}",
                                    name=f"z{i}_{l}")
                      for i in range(2 if opta else 4)]
                nc.vector.memset(ring[:, 1, :, :, CH - 1], 0.0)
                project(l, 0, 0)
                with tc.For_i(0, T_, 2 * CH) as iv:
                    for half in range(2):
                        s, ps = half, 1 - half
                        t0 = iv + half * CH
                        scan_chunk(l, s, ps, zs)
                        writeout(l, s, t0)
                        project(l, iv + (half + 1) * CH, ps)

        reps = int(os.environ.get("DEEPESN_REPS", "1"))
        if reps > 1:
            # benchmarking aid: repeat the whole (idempotent) kernel on
            # device so per-run time can be separated from dispatch cost
            with tc.For_i(0, reps, 1):
                whole_kernel()
        else:
            whole_kernel()

    nc.compile()
    nsplit = split_excess_waits(nc)
    if os.environ.get("DEEPESN_DEBUG"):
        print(f"split_excess_waits: inserted {nsplit} NoOp wait carriers")
    return nc


def _get_nc(T_, mm_fp16):
    key = (T_, mm_fp16, os.environ.get("DEEPESN_REPS", "1"), os.environ.get("DEEPESN_NOCHAIN", "0"), os.environ.get("DEEPESN_ACT1", "0"), os.environ.get("DEEPESN_LDWOPT", "0"), os.environ.get("DEEPESN_OPTA", "0"))
    if key not in _cache:
        _cache[key] = _build(T_, mm_fp16)
    return _cache[key]


def _prepare_in_maps(T_, x, Win0, Wrec0, b0, Win1, Wrec1, b1, Win2, Wrec2,
                     b2):
    x = np.ascontiguousarray(np.asarray(x, dtype=np.float32)[:, :T_])
    pad = np.zeros((B, CH, I), np.float32)
    xp = np.concatenate([x, pad], axis=1)  # [B, T+CH, I]
    ident = np.eye(P, dtype=np.float32)
    weights = {
        "Win0": Win0, "Wrec0": Wrec0, "b0": b0,
        "Win1": Win1, "Wrec1": Wrec1, "b1": b1,
        "Win2": Win2, "Wrec2": Wrec2, "b2": b2,
    }
    weights = {k: np.ascontiguousarray(np.asarray(v, dtype=np.float32))
               for k, v in weights.items()}
    in_maps = []
    for c in range(NCORES):
        m = dict(weights)
        m["x"] = np.ascontiguousarray(xp[c * BL:(c + 1) * BL])
        m["ident"] = ident
        in_maps.append(m)
    return in_maps


def kernel(x, Win0, Wrec0, b0, Win1, Wrec1, b1, Win2, Wrec2, b2):
    from concourse.bass_utils import run_bass_kernel_spmd

    T_ = int(os.environ.get("DEEPESN_T", x.shape[1]))
    mm_fp16 = os.environ.get("DEEPESN_FP16", "0") == "1"
    nc = _get_nc(T_, mm_fp16)
    in_maps = _prepare_in_maps(T_, x, Win0, Wrec0, b0, Win1, Wrec1, b1,
                               Win2, Wrec2, b2)

    res = run_bass_kernel_spmd(nc, in_maps, core_ids=list(range(NCORES)))
    kernel.last_exec_time_ns = res.exec_time_ns
    kernel.last_results = res
    return np.concatenate([res.results[c]["out"] for c in range(NCORES)],
                          axis=0)


kernel.last_exec_time_ns = None

